# revision 2
# baseline (speedup 1.0000x reference)
"""Trainium2 Bass kernel for masked multi-adapter LoRA (moe_routing).

Computes out = result + ((x @ A_cat) * onehot_mask) @ B_cat  where
A_cat [H, 128] stacks the 8 adapters' shrink matrices along the rank dim and
B_cat [128, O] stacks the expand matrices.  Since each token's one-hot mask
zeroes every rank column except its own adapter's 16, this equals the
reference's per-adapter masked update loop exactly (masked terms add 0.0).

Sharding: data-parallel over tokens, T=8192 -> 1024 tokens per core x 8 cores.
Each core runs an identical program (SPMD) on its token shard with the small
adapter stacks replicated.

This problem is HBM-bandwidth bound (3 full [T,4096] tensor passes: read x,
read result, write out).  All DRAM I/O is bf16 (halving traffic vs fp32) and
x is pre-transposed token->H-major on the host so no on-chip transposes are
needed; matmuls accumulate in fp32 PSUM.  Max rel err vs the fp32 reference
is ~2e-3 (bf16 quantization), well inside the 2e-2 gate.

Per-core pipeline (2 token superblocks of 512 for DMA/compute overlap):
  - xT chunks [128 h-part, 8, 512 tok] bf16 DMA'd directly (host transposed).
  - shrink: VT[128rc, 512tok] accumulated in fp32 PSUM over 32 H-chunks with
    A_cat chunks as the stationary operand.
  - mask: one DVE multiply against the host-built one-hot mask (transposed
    layout [128rc, tok]) drains PSUM -> SBUF as bf16.
  - expand: VmT token-slices become the stationary operand; B_cat streams.
  - result tiles are added on DVE (in place, bf16 out) and stored back.
"""

import numpy as np
from contextlib import ExitStack

import ml_dtypes

import concourse.bass as bass
import concourse.mybir as mybir
import concourse.tile as tile
from concourse import bacc
from concourse.bass_utils import run_bass_kernel_spmd

# problem shape (hardcoded per harness contract)
T, H, R, O, NA = 8192, 4096, 16, 4096, 8
NCORES = 8
TS = T // NCORES            # tokens per core = 1024
P = 128
RC = NA * R                 # concatenated rank dim = 128
KC = H // P                 # 32 H-chunks
SB = 512                    # superblock tokens (PSUM bank free-dim)
NSB = TS // SB              # 2 superblocks per core
G = SB // P                 # 4 token tiles per superblock
NJ = O // 512               # 8 expand column chunks
NQ = 4                      # x DMA groups per superblock
KQ = KC // NQ               # 8 H-chunks per x DMA group

F32 = mybir.dt.float32
BF16 = mybir.dt.bfloat16
BF16NP = ml_dtypes.bfloat16

_BUILT = {}


def _emit(tc, xT, res, a_cat, b_cat, maskT, out, repeats=1):
    nc = tc.nc
    ctx = ExitStack()
    with ctx:
        const = ctx.enter_context(tc.tile_pool(name="const", bufs=1))
        xpool = ctx.enter_context(tc.tile_pool(name="xpool", bufs=2 * NQ))
        vpool = ctx.enter_context(tc.tile_pool(name="vpool", bufs=2))
        rpool = ctx.enter_context(tc.tile_pool(name="rpool", bufs=4))
        vt_ps_pool = ctx.enter_context(tc.tile_pool(name="vt_ps", bufs=2, space="PSUM"))
        u_ps_pool = ctx.enter_context(tc.tile_pool(name="u_ps", bufs=4, space="PSUM"))

        # 4D/3D views
        x4 = xT.rearrange("(s q k p) t -> s q p k t", s=NSB, q=NQ, k=KQ, p=P)
        res3 = res.rearrange("(t p) o -> t p o", p=P)
        out3 = out.rearrange("(t p) o -> t p o", p=P)
        a3 = a_cat.rearrange("(ko p) m -> p ko m", p=P)

        # resident tensors
        a_sb = const.tile([P, KC, P], BF16, name="a_sb")
        nc.sync.dma_start(a_sb[:], a3)
        b_sb = const.tile([P, O], BF16, name="b_sb")
        nc.sync.dma_start(b_sb[:], b_cat)
        m_sb = const.tile([P, TS], BF16, name="m_sb")
        nc.sync.dma_start(m_sb[:], maskT)

        for rep in range(repeats):
            # stream x in (8 DMAs per repeat; 1 KiB lines)
            xg = [[None] * NQ for _ in range(NSB)]
            for s in range(NSB):
                for q in range(NQ):
                    xt = xpool.tile([P, KQ, SB], BF16, name=f"xg_{rep}_{s}_{q}",
                                    tag="xg")
                    nc.sync.dma_start(xt[:], x4[s, q])
                    xg[s][q] = xt

            for s in range(NSB):
                # shrink: VT[rc, tok] accumulated over 32 H-chunks
                vt_ps = vt_ps_pool.tile([P, SB], F32, name=f"vt_{rep}_{s}", tag="vt")
                for q in range(NQ):
                    for k in range(KQ):
                        ko = q * KQ + k
                        nc.tensor.matmul(
                            vt_ps[:], a_sb[:, ko], xg[s][q][:, k],
                            start=(ko == 0), stop=(ko == KC - 1),
                        )

                # mask (drains PSUM -> SBUF, downcast to bf16)
                vmT = vpool.tile([P, SB], BF16, name=f"vmT_{rep}_{s}", tag="vmT")
                nc.vector.tensor_tensor(
                    vmT[:], vt_ps[:], m_sb[:, s * SB:(s + 1) * SB],
                    mybir.AluOpType.mult,
                )

                # expand + result add + store, one token tile at a time
                for g in range(G):
                    gg = s * G + g
                    r_sb = rpool.tile([P, O], BF16, name=f"r_{rep}_{gg}", tag="r")
                    nc.sync.dma_start(r_sb[:], res3[gg])
                    for j in range(NJ):
                        u_ps = u_ps_pool.tile([P, 512], F32, name=f"u_{rep}_{gg}_{j}",
                                              tag="u")
                        nc.tensor.matmul(
                            u_ps[:], vmT[:, g * P:(g + 1) * P],
                            b_sb[:, j * 512:(j + 1) * 512],
                            start=True, stop=True,
                        )
                        nc.vector.tensor_tensor(
                            r_sb[:, j * 512:(j + 1) * 512], u_ps[:],
                            r_sb[:, j * 512:(j + 1) * 512],
                            mybir.AluOpType.add,
                        )
                    nc.sync.dma_start(out3[gg], r_sb[:])


def build(repeats=1):
    """Build + compile the per-core Bass program (shared by all 8 cores)."""
    nc = bacc.Bacc("TRN2", target_bir_lowering=False, debug=False,
                   num_devices=NCORES)
    xT = nc.dram_tensor("xT", [NSB * H, SB], BF16, kind="ExternalInput").ap()
    res = nc.dram_tensor("res", [TS, O], BF16, kind="ExternalInput").ap()
    a_cat = nc.dram_tensor("a_cat", [H, RC], BF16, kind="ExternalInput").ap()
    b_cat = nc.dram_tensor("b_cat", [RC, O], BF16, kind="ExternalInput").ap()
    maskT = nc.dram_tensor("maskT", [RC, TS], BF16, kind="ExternalInput").ap()
    out = nc.dram_tensor("out", [TS, O], BF16, kind="ExternalOutput").ap()

    with tile.TileContext(nc) as tc:
        _emit(tc, xT, res, a_cat, b_cat, maskT, out, repeats=repeats)
    nc.compile()
    return nc


def make_in_maps(result, x, lora_a, lora_b, adapter_indices):
    result = np.asarray(result, dtype=np.float32)
    x = np.asarray(x, dtype=np.float32)
    lora_a = np.asarray(lora_a, dtype=np.float32)
    lora_b = np.asarray(lora_b, dtype=np.float32)
    idx = np.asarray(adapter_indices, dtype=np.int32)

    a_cat = np.ascontiguousarray(
        lora_a.transpose(1, 0, 2).reshape(H, RC)).astype(BF16NP)
    b_cat = np.ascontiguousarray(lora_b.reshape(RC, O)).astype(BF16NP)
    c16 = (np.arange(RC) // R).astype(np.int32)

    in_maps = []
    for c in range(NCORES):
        sl = slice(c * TS, (c + 1) * TS)
        mT = (idx[sl][None, :] == c16[:, None]).astype(BF16NP)
        # token-major -> H-major, grouped [s, ko, p, t] to match the kernel's
        # (s q k p) row factorization with ko = q*KQ + k
        xt = x[sl].astype(BF16NP).T                     # [H, TS]
        xt = np.ascontiguousarray(
            xt.reshape(KC, P, NSB, SB).transpose(2, 0, 1, 3).reshape(NSB * H, SB)
        )
        in_maps.append({
            "xT": xt,
            "res": np.ascontiguousarray(result[sl]).astype(BF16NP),
            "a_cat": a_cat,
            "b_cat": b_cat,
            "maskT": np.ascontiguousarray(mT),
        })
    return in_maps


def kernel(result, x, lora_a, lora_b, adapter_indices):
    in_maps = make_in_maps(result, x, lora_a, lora_b, adapter_indices)
    if "nc" not in _BUILT:
        _BUILT["nc"] = build()
    res = run_bass_kernel_spmd(_BUILT["nc"], in_maps, core_ids=list(range(NCORES)))
    return np.concatenate(
        [np.asarray(res.results[c]["out"], dtype=np.float32) for c in range(NCORES)],
        axis=0,
    )


if __name__ == "__main__":
    rng = np.random.default_rng(0)
    inputs = {
        "result": rng.standard_normal((T, O), dtype=np.float32),
        "x": rng.standard_normal((T, H), dtype=np.float32),
        "lora_a": rng.standard_normal((NA, H, R), dtype=np.float32),
        "lora_b": rng.standard_normal((NA, R, O), dtype=np.float32),
        "adapter_indices": rng.integers(0, NA, size=(T,), dtype=np.int32),
    }
    out = kernel(**inputs)
    print("kernel output:", out.shape, out.dtype)
